# revision 3
# baseline (speedup 1.0000x reference)
"""MoE layer (top-2 of 8 experts, selection shared across tokens) on 8 TRN2 cores.

Math (faithful to the reference):
    gates = softmax(x @ W_gate + b_gate)          [N, 8]
    idx0  = top-2 expert indices of token 0       [2]
    s     = per-token top-2 gate VALUES (desc)    [N, 2]
    out   = s0 * (x @ W[A] + b[A]) + s1 * (x @ W[B] + b[B])

Strategy: gating + top-2 is 0.2% of the FLOPs -> computed on host.  The two
active expert matmuls (275 GFLOP) are data-parallel sharded over tokens across
8 cores; expert weights are replicated.  Matmuls run in fp16 (values are small,
so fp16 range is safe and its 10-bit mantissa keeps rel-err ~3e-4),
accumulating fp32 in PSUM.

Perf notes (v2, from the 489us trace of v1):
  - steady-state MM cadence was already at the N=512 streaming floor
    (215.8ns = 512/2.4GHz + NX), so all the waste was head (17.8us before
    the first MM: 2MB of bias constants DMA'd ahead of W/x), cold-HAM
    (first ~23 MMs at 1.2GHz), DMA-paced first psum group, and tail
    (10.8us: epilogue + SWDGE out DMA exposed after the last MM).
  - v2: x is resident in SBUF (loaded once, 8MB), first W block is only
    256 cols wide so the first group is DMA-paced for ~8us not ~17, dummy
    warm-up matmuls run during the DMA fill to pre-trip the HAM clock gate,
    constants load after the critical first block, and the last group
    reorders pa/pb and stores over the idle HWDGE queues.
"""

import functools

import numpy as np

import concourse.bass as bass
import concourse.mybir as mybir
import concourse.tile as tile
from concourse import bacc
from concourse.bass_utils import run_bass_kernel_spmd

N_CORES = 8
N, D_IN, D_HID = 16384, 2048, 2048
NT = N // N_CORES            # tokens per core
KP = 128                     # contraction chunk = partition dim
KCH = D_IN // KP             # 16 K-chunks
# output column blocks; narrow blocks first so the cold-start psum group is
# gated on 2MB of W DMA instead of 4MB
NB_WIDTHS = (256, 256, 512, 512, 512)
assert sum(NB_WIDTHS) == D_HID
TQ = 256                     # token slice per resident-x tile
NQ = NT // TQ                # 8 slices
MPQ = TQ // 128              # m-tiles per slice
N_DUMMY = 10                 # HAM warm-up matmuls issued while DMA fills

F32 = mybir.dt.float32
FP16 = mybir.dt.float16

# The PE streams one moving-operand column per cycle regardless of dtype, but
# 16-bit operands keep the (FWL) weight load fully hidden (97ns vs 187ns) and
# halve DMA.  fp16 (10 mantissa bits) beats bf16 ~8x on accuracy at identical
# speed, and the value ranges here (|x|<~6, |W|<~0.12) are safely inside
# fp16 range.  PSUM accumulates fp32; the per-token top-2 scores applied in
# the epilogue stay fp32.
W_DT = FP16
X_DT = FP16

# Filled by test harness inspection: last BassKernelResults from a run.
LAST_RESULT = None


@functools.lru_cache(maxsize=1)
def _build():
    nc = bacc.Bacc("TRN2", target_bir_lowering=False, debug=False)
    xT = nc.dram_tensor("xT", [D_IN, NT], X_DT, kind="ExternalInput")
    wa = nc.dram_tensor("wa", [D_IN, D_HID], W_DT, kind="ExternalInput")
    wb = nc.dram_tensor("wb", [D_IN, D_HID], W_DT, kind="ExternalInput")
    # bias pre-replicated across partitions on host: brep[p, e, o] = b_sel[e, o]
    brep = nc.dram_tensor("brep", [128, 2, D_HID], F32, kind="ExternalInput")
    # per-token scores pre-arranged on host, partition-major:
    # sC[p, m, s] = top2_score[m*128 + p, s]
    sC = nc.dram_tensor("sC", [128, NT // 128, 2], F32, kind="ExternalInput")
    out = nc.dram_tensor("out", [NT, D_HID], F32, kind="ExternalOutput")

    MULT = mybir.AluOpType.mult
    ADD = mybir.AluOpType.add

    with tile.TileContext(nc) as tc:
        with (
            tc.tile_pool(name="cst", bufs=1) as cst,
            tc.tile_pool(name="wp", bufs=2) as wp,
            tc.tile_pool(name="xr", bufs=1) as xr,
            tc.tile_pool(name="ep", bufs=2) as ep,
            tc.tile_pool(name="ps", bufs=3, space=bass.MemorySpace.PSUM) as ps,
        ):
            # ── HAM warm-up ───────────────────────────────────────────────
            # The PE clock-gate defaults to 1.2GHz and only reaches 2.4GHz
            # after ~3.4us of sustained PE activity.  The first ~4us of the
            # kernel are DMA-fill anyway, so burn them on dummy matmuls into
            # a scratch psum bank: by the time real operands land, the PE is
            # warm.  dm is memset first so CoreSim sees no uninitialized read.
            dm = cst.tile([128, 512], W_DT, tag="dm")
            nc.vector.memset(dm[:], 0.0)
            dps = ps.tile([128, 512], F32, tag="dummy", bufs=1)
            for _ in range(N_DUMMY):
                nc.tensor.matmul(dps[:], dm[:, 0:128], dm[:], start=True, stop=True)

            # ── DMA plan ─────────────────────────────────────────────────
            # sync + scalar are pure DMA-issue queues (no compute on either,
            # so a dma_start blocked on a tile-slot semaphore never stalls
            # math).  Order = strict critical path:
            #   1. x(q0) + W(block0) interleaved k-major on both queues
            #      (the first psum group is gated on exactly these bytes)
            #   2. sC (16KB) + the block-0 slice of brep (the first epilogue
            #      needs them at ~+7us; psum-bank runway covers the slack)
            #   3. remaining x slices q=1..7 (x is RESIDENT: loaded once)
            #   4. rest of brep, then per-block W as the loop reaches it.
            x_t = {}
            w_t = {}

            def q_eng(i):
                return nc.sync if i % 2 == 0 else nc.scalar

            def load_x(q, k):
                t = xr.tile([KP, TQ], X_DT, tag=f"x{k}_{q}")
                q_eng(k + q).dma_start(
                    t[:], xT[k * KP:(k + 1) * KP, q * TQ:(q + 1) * TQ]
                )
                return t

            def load_w(k, e, wd, nb_sl):
                t = wp.tile([KP, nb_sl.stop - nb_sl.start], W_DT, tag=f"w{e}_{k}")
                q_eng(k + e).dma_start(t[:], wd[k * KP:(k + 1) * KP, nb_sl])
                return t

            nb0_sl = slice(0, NB_WIDTHS[0])
            for k in range(KCH):
                x_t[k, 0] = load_x(0, k)
                for e, wd in enumerate((wa, wb)):
                    w_t[e, k] = load_w(k, e, wd, nb0_sl)

            sC_sb = cst.tile([128, NT // 128, 2], F32)
            nc.sync.dma_start(sC_sb[:], sC[:])
            brep_sb = cst.tile([128, 2, D_HID], F32)
            nc.scalar.dma_start(brep_sb[:, :, nb0_sl], brep[:, :, nb0_sl])

            for q in range(1, NQ):
                for k in range(KCH):
                    x_t[k, q] = load_x(q, k)

            nc.sync.dma_start(
                brep_sb[:, :, NB_WIDTHS[0]:], brep[:, :, NB_WIDTHS[0]:]
            )

            # ── main loop ────────────────────────────────────────────────
            nb_off = 0
            for nb, nbw in enumerate(NB_WIDTHS):
                nb_sl = slice(nb_off, nb_off + nbw)
                nb_off += nbw
                if nb > 0:
                    for k in range(KCH):
                        for e, wd in enumerate((wa, wb)):
                            w_t[e, k] = load_w(k, e, wd, nb_sl)
                for q in range(NQ):
                    for mi in range(MPQ):
                        mg = q * MPQ + mi
                        last = (
                            nb == len(NB_WIDTHS) - 1
                            and q == NQ - 1
                            and mi == MPQ - 1
                        )
                        pa = ps.tile([128, 512], F32, tag="pa", name="pa")[:, :nbw]
                        pb = ps.tile([128, 512], F32, tag="pb", name="pb")[:, :nbw]
                        if last:
                            # pa finishes 16 MMs early so its epilogue half
                            # overlaps pb's matmuls -> shorter exposed tail
                            for e, pp in ((0, pa), (1, pb)):
                                for k in range(KCH):
                                    nc.tensor.matmul(
                                        pp[:], x_t[k, q][:, bass.ts(mi, 128)],
                                        w_t[e, k][:],
                                        start=(k == 0), stop=(k == KCH - 1),
                                    )
                        else:
                            for k in range(KCH):
                                xk = x_t[k, q][:, bass.ts(mi, 128)]
                                nc.tensor.matmul(
                                    pa[:], xk, w_t[0, k][:],
                                    start=(k == 0), stop=(k == KCH - 1),
                                )
                                nc.tensor.matmul(
                                    pb[:], xk, w_t[1, k][:],
                                    start=(k == 0), stop=(k == KCH - 1),
                                )
                        s0 = sC_sb[:, mg, 0:1]
                        s1 = sC_sb[:, mg, 1:2]
                        # epilogue on DVE: out = s0*(pa+bA) + s1*(pb+bB)
                        # (each op reads at most one PSUM input)
                        u = ep.tile([128, 512], F32, tag="u", name="u")[:, :nbw]
                        nc.vector.tensor_add(u[:], pa[:], brep_sb[:, 0, nb_sl])
                        t1 = ep.tile([128, 512], F32, tag="t1", name="t1")[:, :nbw]
                        nc.vector.tensor_scalar_mul(t1[:], u[:], s0)
                        v = ep.tile([128, 512], F32, tag="v", name="v")[:, :nbw]
                        nc.vector.tensor_add(v[:], pb[:], brep_sb[:, 1, nb_sl])
                        o = ep.tile([128, 512], F32, tag="o", name="o")[:, :nbw]
                        nc.vector.scalar_tensor_tensor(
                            o[:], v[:], s1, t1[:], op0=MULT, op1=ADD
                        )
                        m_sl = bass.ts(mg, 128)
                        if last:
                            # the fast queues are idle by now; halve + split
                            # so the exposed store is ~0.4us not ~3us
                            h = nbw // 2
                            nc.sync.dma_start(
                                out[m_sl, nb_sl.start:nb_sl.start + h],
                                o[:, 0:h],
                            )
                            nc.scalar.dma_start(
                                out[m_sl, nb_sl.start + h:nb_sl.stop],
                                o[:, h:nbw],
                            )
                        else:
                            nc.gpsimd.dma_start(out[m_sl, nb_sl], o[:])

    nc.compile()
    return nc


def _host_gating(x, W_gate, b_gate):
    logits = x @ W_gate + b_gate                       # [N, 8] fp32
    m = logits.max(axis=1, keepdims=True)
    e = np.exp(logits - m)
    gates = e / e.sum(axis=1, keepdims=True)
    idx0 = np.argsort(-gates[0], kind="stable")[:2]    # token-0 top-2 experts
    scores = -np.sort(-gates, axis=1)[:, :2]           # per-token top-2 values
    return idx0, np.ascontiguousarray(scores)


def kernel(x, W_experts, b_experts, W_gate, b_gate):
    global LAST_RESULT
    x = np.ascontiguousarray(np.asarray(x, dtype=np.float32))
    W_experts = np.asarray(W_experts, dtype=np.float32)
    b_experts = np.asarray(b_experts, dtype=np.float32)
    W_gate = np.asarray(W_gate, dtype=np.float32)
    b_gate = np.asarray(b_gate, dtype=np.float32)

    idx0, scores = _host_gating(x, W_gate, b_gate)
    w_np_dt = mybir.dt.np(W_DT)
    x_np_dt = mybir.dt.np(X_DT)
    wa = np.ascontiguousarray(W_experts[idx0[0]]).astype(w_np_dt)  # [D_IN, D_HID]
    wb = np.ascontiguousarray(W_experts[idx0[1]]).astype(w_np_dt)
    brep = np.ascontiguousarray(
        np.broadcast_to(b_experts[idx0][None], (128, 2, D_HID))
    ).astype(np.float32)

    xT_full = np.ascontiguousarray(x.astype(x_np_dt).T)            # [D_IN, N]

    nc = _build()
    in_maps = []
    for c in range(N_CORES):
        sl = slice(c * NT, (c + 1) * NT)
        in_maps.append(
            {
                "xT": np.ascontiguousarray(xT_full[:, sl]),
                "wa": wa,
                "wb": wb,
                "brep": brep,
                "sC": np.ascontiguousarray(
                    scores[sl].reshape(NT // 128, 128, 2).transpose(1, 0, 2)
                ),
            }
        )

    res = run_bass_kernel_spmd(nc, in_maps, list(range(N_CORES)))
    LAST_RESULT = res
    return np.concatenate([r["out"] for r in res.results], axis=0)


# revision 5
# speedup vs baseline: 1.0654x; 1.0654x over previous
"""MoE layer (top-2 of 8 experts, selection shared across tokens) on 8 TRN2 cores.

Math (faithful to the reference):
    gates = softmax(x @ W_gate + b_gate)          [N, 8]
    idx0  = top-2 expert indices of token 0       [2]
    s     = per-token top-2 gate VALUES (desc)    [N, 2]
    out   = s0 * (x @ W[A] + b[A]) + s1 * (x @ W[B] + b[B])

Strategy: gating + top-2 is 0.2% of the FLOPs -> computed on host.  The two
active expert matmuls (275 GFLOP) are data-parallel sharded over tokens across
8 cores; expert weights are replicated.  Matmuls run in fp16 (values are small,
so fp16 range is safe and its 10-bit mantissa keeps rel-err ~3e-4),
accumulating fp32 in PSUM.

Perf notes (v3):
  - steady-state MM cadence is at the N=512 streaming floor (215.8ns =
    512/2.4GHz + NX issue), so the only wins left are head/tail/gap removal.
  - DMA throughput is descriptor-run-bound: [128,256]-fp16 tiles (512B runs)
    move at ~140GB/s while 1-2MB transfers with 8KB runs hit ~340GB/s.  So
    the host pre-blocks x and W into per-transfer-contiguous DRAM layouts and
    every load is one big DMA: x = 1MB per 256-token slice (resident in SBUF,
    loaded once), W = 2MB per (expert, 512-col block).
  - the PE clock-gate (HAM) starts at 1.2GHz and needs ~3.4us of activity to
    reach 2.4GHz: dummy matmuls on a scratch psum bank run during the DMA
    fill so real work starts warm.
  - critical-path DMA order: x(q0) + W(block0) in k-halves on both HWDGE
    queues, then sC + the block-0 bias slice, then remaining x, then W(nb+1)
    prefetched at each block start.  Output stores ride the same two HWDGE
    queues (no SWDGE at all), alternating per group.
  - last group runs expert-A's 16 matmuls before expert-B's so half the
    epilogue overlaps the tail matmuls; its store is split across both queues.
"""

import functools

import numpy as np

import concourse.bass as bass
import concourse.mybir as mybir
import concourse.tile as tile
from concourse import bacc
from concourse.bass_utils import run_bass_kernel_spmd

N_CORES = 8
N, D_IN, D_HID = 16384, 2048, 2048
NT = N // N_CORES            # tokens per core
KP = 128                     # contraction chunk = partition dim
KCH = D_IN // KP             # 16 K-chunks
NB = 512                     # output column block (1 PSUM bank of fp32)
NBLK = D_HID // NB           # 4 output blocks
TQ = 256                     # token slice per resident-x tile
NQ = NT // TQ                # 8 slices
MPQ = TQ // 128              # m-tiles per slice
N_DUMMY = 9                  # HAM warm-up matmuls issued while DMA fills

F32 = mybir.dt.float32
FP16 = mybir.dt.float16

W_DT = FP16
X_DT = FP16

# Filled by test harness inspection: last BassKernelResults from a run.
LAST_RESULT = None


@functools.lru_cache(maxsize=1)
def _build():
    nc = bacc.Bacc("TRN2", target_bir_lowering=False, debug=False)
    # host-blocked layouts: each (q) / (e, nb) slice is contiguous in DRAM
    # xb[q, p, k*TQ+j] = x.T[k*128+p, q*TQ+j]
    xb = nc.dram_tensor("xb", [NQ, 128, KCH * TQ], X_DT, kind="ExternalInput")
    # wb_[e, nb, p, k*NB+j] = W_e[k*128+p, nb*NB+j]
    wb_ = nc.dram_tensor(
        "wb", [2, NBLK, 128, KCH * NB], W_DT, kind="ExternalInput"
    )
    # bias pre-replicated across partitions on host: brep[p, e, o] = b_sel[e, o]
    brep = nc.dram_tensor("brep", [128, 2, D_HID], F32, kind="ExternalInput")
    # per-token scores pre-arranged on host, partition-major:
    # sC[p, m, s] = top2_score[m*128 + p, s]
    sC = nc.dram_tensor("sC", [128, NT // 128, 2], F32, kind="ExternalInput")
    out = nc.dram_tensor("out", [NT, D_HID], F32, kind="ExternalOutput")

    MULT = mybir.AluOpType.mult
    ADD = mybir.AluOpType.add
    HALF = KCH // 2 * TQ     # free-dim cols in half the k-chunks of an x slice
    WHALF = KCH // 2 * NB

    with tile.TileContext(nc) as tc:
        with (
            tc.tile_pool(name="cst", bufs=1) as cst,
            tc.tile_pool(name="wp", bufs=2) as wp,
            tc.tile_pool(name="xr", bufs=1) as xr,
            tc.tile_pool(name="ep", bufs=2) as ep,
            tc.tile_pool(name="ps", bufs=3, space=bass.MemorySpace.PSUM) as ps,
        ):
            # HAM warm-up: dummy matmuls into a scratch psum bank while the
            # first real operands stream in, so real MMs start at 2.4GHz.
            dm = cst.tile([128, 512], W_DT, tag="dm")
            nc.vector.memset(dm[:], 0.0)
            dps = ps.tile([128, 512], F32, tag="dummy", bufs=1)
            for _ in range(N_DUMMY):
                nc.tensor.matmul(dps[:], dm[:, 0:128], dm[:], start=True, stop=True)

            x_t = {}
            w_t = {}

            # ── critical-path DMA ────────────────────────────────────────
            # sync queue: x(q0).k0-7, Wa.k0-7, x(q0).k8-15, Wa.k8-15, ...
            # scalar   : Wb.k0-7, Wb.k8-15, sC, brep(block0), ...
            x0 = xr.tile([128, KCH * TQ], X_DT, tag="x0", name="x0")
            wa0 = wp.tile([128, KCH * NB], W_DT, tag="w0", name="wa0")
            wb0 = wp.tile([128, KCH * NB], W_DT, tag="w1", name="wb0")
            nc.sync.dma_start(x0[:, 0:HALF], xb[0, :, 0:HALF])
            nc.scalar.dma_start(wb0[:, 0:WHALF], wb_[1, 0, :, 0:WHALF])
            nc.sync.dma_start(wa0[:, 0:WHALF], wb_[0, 0, :, 0:WHALF])
            nc.scalar.dma_start(wb0[:, WHALF:], wb_[1, 0, :, WHALF:])
            nc.sync.dma_start(x0[:, HALF:], xb[0, :, HALF:])
            nc.sync.dma_start(wa0[:, WHALF:], wb_[0, 0, :, WHALF:])
            x_t[0] = x0
            w_t[0, 0] = wa0
            w_t[1, 0] = wb0

            sC_sb = cst.tile([128, NT // 128, 2], F32)
            nc.scalar.dma_start(sC_sb[:], sC[:])
            brep_sb = cst.tile([128, 2, D_HID], F32)
            nc.scalar.dma_start(brep_sb[:, :, 0:NB], brep[:, :, 0:NB])

            for q in range(1, NQ):
                t = xr.tile([128, KCH * TQ], X_DT, tag=f"x{q}", name=f"x{q}")
                (nc.sync if q % 2 == 1 else nc.scalar).dma_start(t[:], xb[q])
                x_t[q] = t

            nc.sync.dma_start(brep_sb[:, :, NB:], brep[:, :, NB:])

            def prefetch_w(nb):
                for e in range(2):
                    t = wp.tile(
                        [128, KCH * NB], W_DT, tag=f"w{e}", name=f"w{e}_{nb}"
                    )
                    (nc.sync if e == 0 else nc.scalar).dma_start(
                        t[:], wb_[e, nb]
                    )
                    w_t[e, nb] = t

            prefetch_w(1)

            # ── main loop ────────────────────────────────────────────────
            for nb in range(NBLK):
                nb_sl = bass.ts(nb, NB)
                if nb + 2 <= NBLK - 1:
                    prefetch_w(nb + 2)
                wa_c = w_t[0, nb]
                wb_c = w_t[1, nb]
                for q in range(NQ):
                    for mi in range(MPQ):
                        mg = q * MPQ + mi
                        last = nb == NBLK - 1 and mg == NQ * MPQ - 1
                        pa = ps.tile([128, NB], F32, tag="pa", name="pa")
                        pb = ps.tile([128, NB], F32, tag="pb", name="pb")
                        xq = x_t[q]

                        def xs(k):
                            return xq[:, k * TQ + mi * 128:k * TQ + mi * 128 + 128]

                        def ws(w, k):
                            return w[:, k * NB:(k + 1) * NB]

                        if last:
                            # pa finishes 16 MMs early so its epilogue half
                            # overlaps pb's matmuls -> shorter exposed tail
                            for pp, wc in ((pa, wa_c), (pb, wb_c)):
                                for k in range(KCH):
                                    nc.tensor.matmul(
                                        pp[:], xs(k), ws(wc, k),
                                        start=(k == 0), stop=(k == KCH - 1),
                                    )
                        else:
                            for k in range(KCH):
                                nc.tensor.matmul(
                                    pa[:], xs(k), ws(wa_c, k),
                                    start=(k == 0), stop=(k == KCH - 1),
                                )
                                nc.tensor.matmul(
                                    pb[:], xs(k), ws(wb_c, k),
                                    start=(k == 0), stop=(k == KCH - 1),
                                )
                        s0 = sC_sb[:, mg, 0:1]
                        s1 = sC_sb[:, mg, 1:2]
                        # epilogue on DVE: out = s0*(pa+bA) + s1*(pb+bB)
                        # (each op reads at most one PSUM input)
                        u = ep.tile([128, NB], F32, tag="u", name="u")
                        nc.vector.tensor_add(u[:], pa[:], brep_sb[:, 0, nb_sl])
                        t1 = ep.tile([128, NB], F32, tag="t1", name="t1")
                        nc.vector.tensor_scalar_mul(t1[:], u[:], s0)
                        v = ep.tile([128, NB], F32, tag="v", name="v")
                        nc.vector.tensor_add(v[:], pb[:], brep_sb[:, 1, nb_sl])
                        o = ep.tile([128, NB], F32, tag="o", name="o")
                        nc.vector.scalar_tensor_tensor(
                            o[:], v[:], s1, t1[:], op0=MULT, op1=ADD
                        )
                        m_sl = bass.ts(mg, 128)
                        if last:
                            # both queues are idle by now; split the store
                            h = NB // 2
                            nc.sync.dma_start(
                                out[m_sl, nb * NB:nb * NB + h], o[:, 0:h]
                            )
                            nc.scalar.dma_start(
                                out[m_sl, nb * NB + h:(nb + 1) * NB], o[:, h:]
                            )
                        else:
                            eng = nc.sync if mg % 2 == 0 else nc.scalar
                            eng.dma_start(out[m_sl, nb_sl], o[:])

    nc.compile()
    return nc


def _host_gating(x, W_gate, b_gate):
    logits = x @ W_gate + b_gate                       # [N, 8] fp32
    m = logits.max(axis=1, keepdims=True)
    e = np.exp(logits - m)
    gates = e / e.sum(axis=1, keepdims=True)
    idx0 = np.argsort(-gates[0], kind="stable")[:2]    # token-0 top-2 experts
    scores = -np.sort(-gates, axis=1)[:, :2]           # per-token top-2 values
    return idx0, np.ascontiguousarray(scores)


def kernel(x, W_experts, b_experts, W_gate, b_gate):
    global LAST_RESULT
    x = np.ascontiguousarray(np.asarray(x, dtype=np.float32))
    W_experts = np.asarray(W_experts, dtype=np.float32)
    b_experts = np.asarray(b_experts, dtype=np.float32)
    W_gate = np.asarray(W_gate, dtype=np.float32)
    b_gate = np.asarray(b_gate, dtype=np.float32)

    idx0, scores = _host_gating(x, W_gate, b_gate)
    w_np_dt = mybir.dt.np(W_DT)
    x_np_dt = mybir.dt.np(X_DT)

    # blocked W: [2, NBLK, 128, KCH*NB]; wblk[e, nb, p, k*NB+j] = W_e[k*128+p, nb*NB+j]
    w_sel = np.stack([W_experts[idx0[0]], W_experts[idx0[1]]])  # [2, D_IN, D_HID]
    wblk = np.ascontiguousarray(
        w_sel.reshape(2, KCH, 128, NBLK, NB)
        .transpose(0, 3, 2, 1, 4)
        .reshape(2, NBLK, 128, KCH * NB)
    ).astype(w_np_dt)

    brep = np.ascontiguousarray(
        np.broadcast_to(b_experts[idx0][None], (128, 2, D_HID))
    ).astype(np.float32)

    xT_full = x.astype(x_np_dt).T                                  # [D_IN, N]

    nc = _build()
    in_maps = []
    for c in range(N_CORES):
        sl = slice(c * NT, (c + 1) * NT)
        # blocked x: [NQ, 128, KCH*TQ]; xbc[q, p, k*TQ+j] = xT[k*128+p, q*TQ+j]
        xbc = np.ascontiguousarray(
            xT_full[:, sl]
            .reshape(KCH, 128, NQ, TQ)
            .transpose(2, 1, 0, 3)
            .reshape(NQ, 128, KCH * TQ)
        )
        in_maps.append(
            {
                "xb": xbc,
                "wb": wblk,
                "brep": brep,
                "sC": np.ascontiguousarray(
                    scores[sl].reshape(NT // 128, 128, 2).transpose(1, 0, 2)
                ),
            }
        )

    res = run_bass_kernel_spmd(nc, in_maps, list(range(N_CORES)))
    LAST_RESULT = res
    return np.concatenate([r["out"] for r in res.results], axis=0)
